# revision 47
# baseline (speedup 1.0000x reference)
"""Bass/Trainium2 kernel for EnhancedGNNCap message passing (8 NeuronCores).

Strategy v2 (node-sharded, edge-streamed, gather-free):
  - Host: sort edges by dst, shard nodes (and their incoming edges) across
    8 cores, group edges into 128-dst-node windows, pack per-window padded
    128-edge tiles. Host gathers x[src]/x[dst] rows into contiguous
    channel-major bf16 streams (edge-parallel input sharding) so the device
    needs NO dma_gather and NO AllGather.
  - Device edge phase, per 128-edge tile (PSUM [e, oc]):
        pre = x_iT.T@W1i + x_jT.T@W1j + ea_augT.T@W1e_aug      (3 matmuls)
        h   = relu(pre)  (batched 4 tiles per ACT op, bf16)
        A_T[oc, n] += h.T @ S   (S = on-device one-hot of dst offsets)
    S is built once per window by a single DVE/Pool is_equal over broadcast
    views of a [128, T] dst-offset table vs an iota row.
  - W2/b2 are folded into the GRU/gate weights on the host (weight*weight),
    so A goes straight into the node phase.
  - Node phase in fp32r (full f32 accuracy, 1 cycle/col at N=512):
    GRU + gate + LayerNorm in [ch, node] orientation, transpose, write out.
All per-core differences are carried in input data; one SPMD program.
"""

import os
import sys
import types

sys.path.insert(0, "/opt/trn_rl_repo")

import numpy as np


def _install_ntff_hook():
    """Register the axon NTFF profiling hook if the image lacks antenv.axon_hooks."""
    try:
        import antenv
        try:
            import antenv.axon_hooks  # noqa: F401
            return
        except ImportError:
            pass
        m = types.ModuleType("antenv.axon_hooks")
        m._hook = None
        m.set_axon_ntff_profile_hook = lambda h: setattr(m, "_hook", h)
        m.get_axon_ntff_profile_hook = lambda: m._hook
        sys.modules["antenv.axon_hooks"] = m
        antenv.axon_hooks = m
        from trn_agent_boot.trn_boot import _ntff_profile_via_ctypes
        m.set_axon_ntff_profile_hook(_ntff_profile_via_ctypes("/opt/axon/libaxon_pjrt.so"))
    except Exception:
        pass


_install_ntff_hook()

import ml_dtypes  # noqa: E402
import concourse.bass as bass  # noqa: E402
import concourse.bacc as bacc  # noqa: E402
import concourse.mybir as mybir  # noqa: E402
import concourse.tile as tile  # noqa: E402
from concourse.masks import make_identity  # noqa: E402
from concourse.bass_utils import run_bass_kernel_spmd  # noqa: E402

BF = mybir.dt.bfloat16
F32 = mybir.dt.float32
F32R = mybir.dt.float32r
NPBF = ml_dtypes.bfloat16

FULL_CFG = dict(
    n_nodes=50000,
    n_cores=8,
    ch=128,
    edge_dim=7,
    win=128,        # dst nodes per scatter window
    grp=4,          # tiles per relu/psum group (4*128 = 512 psum cols)
    nb=512,         # node-phase group width
)


# --------------------------------------------------------------------------
# host-side preparation: sort/shard/pad edges, build per-core input arrays
# --------------------------------------------------------------------------

def host_prep(x, edge_index, edge_attr, cfg):
    n_nodes = cfg["n_nodes"]
    n_cores = cfg["n_cores"]
    win = cfg["win"]
    ch = cfg["ch"]
    ed = cfg["edge_dim"]
    npc = n_nodes // n_cores            # nodes per core
    n_win = -(-npc // win)              # windows per core

    src = np.asarray(edge_index[0], dtype=np.int64)
    dst = np.asarray(edge_index[1], dtype=np.int64)
    ea = np.asarray(edge_attr, dtype=np.float32)

    order = np.argsort(dst, kind="stable")
    src_s = src[order].astype(np.int64)
    dst_s = dst[order].astype(np.int64)
    ea_s = ea[order]

    deg_full = np.bincount(dst_s, minlength=n_nodes).astype(np.float32)
    x_f = np.asarray(x, dtype=np.float32)
    x_bf = x_f.astype(NPBF)

    core_bounds = np.searchsorted(dst_s, np.arange(n_cores + 1) * npc)

    # per-core per-window edge counts -> shared tile counts TW[w]
    cnt = np.zeros((n_cores, n_win), dtype=np.int64)
    core_data = []
    for c in range(n_cores):
        e0, e1 = core_bounds[c], core_bounds[c + 1]
        d_loc = dst_s[e0:e1] - c * npc
        wid = d_loc // win
        cnt[c] = np.bincount(wid, minlength=n_win)
        core_data.append((e0, e1, d_loc, wid))
    TW = np.maximum(-(-cnt.max(axis=0) // 128), 1)     # tiles per window
    off = np.zeros(n_win + 1, dtype=np.int64)
    off[1:] = np.cumsum(TW)
    T_total = int(off[-1])
    E_slots = T_total * 128

    in_maps = []
    for c in range(n_cores):
        e0, e1, d_loc, wid = core_data[c]
        n_e = e1 - e0
        # rank of each edge within its window (edges are dst-sorted)
        wstart = np.concatenate(([0], np.cumsum(cnt[c])))[:-1]
        rank = np.arange(n_e) - wstart[wid]
        slots = off[wid] * 128 + rank                   # position in stream

        s_c = src_s[e0:e1]
        xiT = np.zeros((ch, E_slots), dtype=NPBF)
        xiT[:, slots] = x_bf[dst_s[e0:e1]].T
        xjT = np.zeros((ch, E_slots), dtype=NPBF)
        xjT[:, slots] = x_bf[s_c].T
        eaT = np.zeros((ed + 1, E_slots), dtype=NPBF)
        eaT[:ed, slots] = ea_s[e0:e1].T.astype(NPBF)
        eaT[ed, slots] = 1.0                            # b1 carrier (pads: 0)

        dflat = np.full(E_slots, -1.0, dtype=np.float32)
        dflat[slots] = (d_loc % win).astype(np.float32)
        dstrel = np.ascontiguousarray(
            dflat.reshape(T_total, 128).T).astype(NPBF)  # [128, T]

        xT = np.ascontiguousarray(x_f[c * npc:(c + 1) * npc].T)  # [ch, npc]
        deg = deg_full[c * npc:(c + 1) * npc].reshape(1, npc)

        in_maps.append(dict(
            xiT=np.ascontiguousarray(xiT), xjT=np.ascontiguousarray(xjT),
            eaT=np.ascontiguousarray(eaT), dstrel=dstrel,
            xT=xT, deg=deg,
        ))

    meta = dict(T_total=T_total, TW=TW, off=off, n_win=n_win, npc=npc)
    return in_maps, meta


def prep_weights(W1, b1, W2, b2, Wg, bg, W_ih, b_ih, W_hh, b_hh, gamma, beta, cfg):
    ch, ed = cfg["ch"], cfg["edge_dim"]
    W1 = np.asarray(W1, np.float32)
    W2 = np.asarray(W2, np.float32)
    b2 = np.asarray(b2, np.float32).reshape(1, ch)
    Wg = np.asarray(Wg, np.float32)
    WihT = np.ascontiguousarray(np.asarray(W_ih, np.float32).T)   # [ch(out), 3ch]
    WhhT = np.ascontiguousarray(np.asarray(W_hh, np.float32).T)   # [ch, 3ch]
    bih = np.asarray(b_ih, np.float32).reshape(3, ch)
    bhh = np.asarray(b_hh, np.float32).reshape(3, ch)

    W1e_aug = np.zeros((ed + 1, ch), dtype=np.float32)
    W1e_aug[:ed] = W1[2 * ch:2 * ch + ed]
    W1e_aug[ed] = np.asarray(b1, np.float32)

    # fold msg_net layer 2 (W2, b2) into the node-phase weights
    WihA = W2 @ WihT                                   # [ch, 3ch]
    dWih = b2 @ WihT                                   # [1, 3ch]
    WgA = W2 @ Wg[ch:2 * ch]                           # [ch, ch]
    dWg = b2 @ Wg[ch:2 * ch]                           # [1, ch]
    Wgx = Wg[0:ch] + Wg[2 * ch:3 * ch]                 # [ch, ch]

    w = dict(
        W1i=W1[0:ch].astype(NPBF),
        W1j=W1[ch:2 * ch].astype(NPBF),
        W1e=W1e_aug.astype(NPBF),
        WihA=np.ascontiguousarray(WihA),
        WhhT=WhhT,
        dWih=np.ascontiguousarray(dWih),
        WgA=np.ascontiguousarray(WgA),
        dWg=np.ascontiguousarray(dWg),
        Wgx=np.ascontiguousarray(Wgx),
        bsum_r=(bih[0] + bhh[0]).reshape(ch, 1).copy(),
        nbsum_z=(-(bih[1] + bhh[1])).reshape(ch, 1).copy(),
        bih_n=bih[2].reshape(ch, 1).copy(),
        bhh_n=bhh[2].reshape(ch, 1).copy(),
        bg_c=np.asarray(bg, np.float32).reshape(ch, 1),
        gam=np.tile(np.asarray(gamma, np.float32).reshape(1, ch), (128, 1)),
        bet=np.tile(np.asarray(beta, np.float32).reshape(1, ch), (128, 1)),
    )
    return w


WSPECS = dict(
    W1i=([128, 128], BF), W1j=([128, 128], BF), W1e=([8, 128], BF),
    WihA=([128, 384], F32R), WhhT=([128, 384], F32R), dWih=([1, 384], F32R),
    WgA=([128, 128], F32R), dWg=([1, 128], F32R), Wgx=([128, 128], F32R),
    bsum_r=([128, 1], F32), nbsum_z=([128, 1], F32),
    bih_n=([128, 1], F32), bhh_n=([128, 1], F32), bg_c=([128, 1], F32),
    gam=([128, 128], F32), bet=([128, 128], F32),
)


# --------------------------------------------------------------------------
# device program
# --------------------------------------------------------------------------

def build_program(cfg, meta):
    ch, ed = cfg["ch"], cfg["edge_dim"]
    n_cores = cfg["n_cores"]
    win, grp, NB = cfg["win"], cfg["grp"], cfg["nb"]
    npc, n_win, T = meta["npc"], meta["n_win"], meta["T_total"]
    TW, off = meta["TW"], meta["off"]
    maxw = int(TW.max())
    has_b2 = bool(cfg.get("has_b2", True))
    AF = mybir.ActivationFunctionType
    OP = mybir.AluOpType

    nc = bacc.Bacc("TRN2", target_bir_lowering=False, debug=False,
                   num_devices=n_cores)

    # ---- I/O ----
    xi_in = nc.dram_tensor("xiT", [ch, T * 128], BF, kind="ExternalInput")
    xj_in = nc.dram_tensor("xjT", [ch, T * 128], BF, kind="ExternalInput")
    ea_in = nc.dram_tensor("eaT", [ed + 1, T * 128], BF, kind="ExternalInput")
    dr_in = nc.dram_tensor("dstrel", [128, T], BF, kind="ExternalInput")
    xT_in = nc.dram_tensor("xT", [ch, npc], F32R, kind="ExternalInput")
    deg_in = nc.dram_tensor("deg", [1, npc], F32R, kind="ExternalInput")
    w_in = {}
    for k, (shp, dt) in WSPECS.items():
        w_in[k] = nc.dram_tensor(k, shp, dt, kind="ExternalInput")
    out_t = nc.dram_tensor("out", [npc, ch], F32, kind="ExternalOutput")

    with tile.TileContext(nc) as tc:
        with (
            tc.tile_pool(name="res", bufs=1) as res,
            tc.tile_pool(name="psum", bufs=1, space="PSUM") as pp,
            tc.tile_pool(name="wk", bufs=2) as wk,
        ):
            # ---------- resident loads ----------
            dr_sb = res.tile([128, T], BF)
            nc.sync.dma_start(out=dr_sb[:], in_=dr_in[:])
            xT_sb = res.tile([ch, npc], F32R)
            nc.sync.dma_start(out=xT_sb[:], in_=xT_in[:])
            deg_sb = res.tile([1, npc], F32R)
            nc.sync.dma_start(out=deg_sb[:], in_=deg_in[:])
            w_sb = {}
            for k, (shp, dt) in WSPECS.items():
                w_sb[k] = res.tile(shp, dt, tag=f"w_{k}", name=f"w_{k}")
                nc.sync.dma_start(out=w_sb[k][:], in_=w_in[k][:])

            # ---------- constants ----------
            iota_row = res.tile([128, 128], BF)
            nc.gpsimd.iota(iota_row[:], pattern=[[1, 128]], base=0,
                           channel_multiplier=0,
                           allow_small_or_imprecise_dtypes=True)
            ident_f = res.tile([128, 128], F32)
            make_identity(nc, ident_f[:])
            eps_col = res.tile([128, 1], F32)
            nc.vector.memset(eps_col[:], 1e-5)

            # aggregated messages, [oc, node], f32r (pre-W2; W2 folded on host)
            aggr_sb = res.tile([ch, npc], F32R)
            # GRU output (pre-LayerNorm), [ch, node] f32
            preo_sb = res.tile([ch, npc], F32)

            # ---------- node-phase group (emitted interleaved) ----------
            n_nb = -(-npc // NB)

            def node_group(j):
                n0 = j * NB
                nn = min(NB, npc - n0)
                ab = aggr_sb[:, n0:n0 + nn]
                xb = xT_sb[:, n0:n0 + nn]
                xbf = xT_sb[:, n0:n0 + nn].bitcast(F32)
                db = deg_sb[:, n0:n0 + nn]

                def gru_mm(psum, wA, wX, wD, c0):
                    last = not (wX or has_b2)
                    nc.tensor.matmul(out=psum[:, :nn],
                                     lhsT=w_sb[wA][:, c0:c0 + ch],
                                     rhs=ab, start=True, stop=last,
                                     skip_group_check=True)
                    if wX is not None:
                        nc.tensor.matmul(out=psum[:, :nn],
                                         lhsT=w_sb[wX][:, c0:c0 + ch],
                                         rhs=xb, start=False,
                                         stop=not has_b2,
                                         skip_group_check=True)
                    if has_b2:
                        nc.tensor.matmul(out=psum[:, :nn],
                                         lhsT=w_sb[wD][:, c0:c0 + ch],
                                         rhs=db, start=False, stop=True,
                                         skip_group_check=True)

                ps_r = pp.tile([128, NB], F32, tag="pre", bufs=2)
                gru_mm(ps_r, "WihA", "WhhT", "dWih", 0)
                r_sb = wk.tile([128, NB], F32, tag="r", bufs=1)
                nc.scalar.activation(out=r_sb[:, :nn], in_=ps_r[:, :nn],
                                     func=AF.Sigmoid, bias=w_sb["bsum_r"][:])

                ps_z = pp.tile([128, NB], F32, tag="pz", bufs=1)
                gru_mm(ps_z, "WihA", "WhhT", "dWih", ch)
                zc_sb = wk.tile([128, NB], F32, tag="zc", bufs=1)
                nc.scalar.activation(out=zc_sb[:, :nn], in_=ps_z[:, :nn],
                                     func=AF.Sigmoid, scale=-1.0,
                                     bias=w_sb["nbsum_z"][:])

                ps_nh = pp.tile([128, NB], F32, tag="pre", bufs=2)
                nc.tensor.matmul(out=ps_nh[:, :nn],
                                 lhsT=w_sb["WhhT"][:, 2 * ch:3 * ch],
                                 rhs=xb, start=True, stop=True,
                                 skip_group_check=True)
                ghn = wk.tile([128, NB], F32, tag="ghn", bufs=1)
                nc.vector.tensor_scalar(out=ghn[:, :nn], in0=ps_nh[:, :nn],
                                        scalar1=w_sb["bhh_n"][:], scalar2=None,
                                        op0=OP.add)

                ps_ni = pp.tile([128, NB], F32, tag="pz", bufs=1)
                nc.tensor.matmul(out=ps_ni[:, :nn],
                                 lhsT=w_sb["WihA"][:, 2 * ch:3 * ch],
                                 rhs=ab, start=True, stop=not has_b2,
                                 skip_group_check=True)
                if has_b2:
                    nc.tensor.matmul(out=ps_ni[:, :nn],
                                     lhsT=w_sb["dWih"][:, 2 * ch:3 * ch],
                                     rhs=db, start=False, stop=True,
                                     skip_group_check=True)

                rgh = wk.tile([128, NB], F32, tag="rgh", bufs=1)
                nc.gpsimd.tensor_tensor(out=rgh[:, :nn], in0=r_sb[:, :nn],
                                        in1=ghn[:, :nn], op=OP.mult)
                nin = wk.tile([128, NB], F32, tag="nin", bufs=1)
                nc.vector.tensor_tensor(out=nin[:, :nn], in0=rgh[:, :nn],
                                        in1=ps_ni[:, :nn], op=OP.add)
                n_sb = wk.tile([128, NB], F32, tag="n", bufs=1)
                nc.scalar.activation(out=n_sb[:, :nn], in_=nin[:, :nn],
                                     func=AF.Tanh, bias=w_sb["bih_n"][:])

                ps_g = pp.tile([128, NB], F32, tag="pg", bufs=1)
                nc.tensor.matmul(out=ps_g[:, :nn],
                                 lhsT=w_sb["WgA"][:], rhs=ab,
                                 start=True, stop=False, skip_group_check=True)
                nc.tensor.matmul(out=ps_g[:, :nn],
                                 lhsT=w_sb["Wgx"][:], rhs=xb,
                                 start=False, stop=not has_b2,
                                 skip_group_check=True)
                if has_b2:
                    nc.tensor.matmul(out=ps_g[:, :nn],
                                     lhsT=w_sb["dWg"][:], rhs=db,
                                     start=False, stop=True,
                                     skip_group_check=True)
                g_sb = wk.tile([128, NB], F32, tag="g", bufs=1)
                nc.scalar.activation(out=g_sb[:, :nn], in_=ps_g[:, :nn],
                                     func=AF.Sigmoid, bias=w_sb["bg_c"][:])

                # out_pre = x + gate*(1-z)*(n - x)
                d_sb = wk.tile([128, NB], F32, tag="d", bufs=1)
                nc.gpsimd.tensor_tensor(out=d_sb[:, :nn], in0=n_sb[:, :nn],
                                        in1=xbf, op=OP.subtract)
                q_sb = wk.tile([128, NB], F32, tag="q", bufs=1)
                nc.gpsimd.tensor_tensor(out=q_sb[:, :nn], in0=g_sb[:, :nn],
                                        in1=zc_sb[:, :nn], op=OP.mult)
                e_sb = wk.tile([128, NB], F32, tag="e", bufs=1)
                nc.gpsimd.tensor_tensor(out=e_sb[:, :nn], in0=q_sb[:, :nn],
                                        in1=d_sb[:, :nn], op=OP.mult)
                nc.gpsimd.tensor_tensor(
                    out=preo_sb[:, n0:n0 + nn],
                    in0=e_sb[:, :nn], in1=xbf, op=OP.add)

            ng = {"next": 0, "drained": 0}

            def dispatch_node_groups(slack=2):
                while ng["next"] < n_nb:
                    j = ng["next"]
                    need_w = -(-min((j + 1) * NB, npc) // win)
                    if ng["drained"] < min(need_w + slack, n_win):
                        return
                    node_group(j)
                    ng["next"] += 1

            # ---------- edge phase ----------
            # scatter matmuls run one relu-group behind the pre matmuls so
            # the in-order PE stream never stalls waiting on ACT
            s_tiles = [None] * n_win

            def build_s(wi):
                twi = int(TW[wi])
                t0i = int(off[wi])
                s_t = wk.tile([128, maxw * 128], BF, tag="s", bufs=4)
                nc.vector.tensor_tensor(
                    out=s_t[:, :twi * 128].rearrange("p (t n) -> p t n", t=twi),
                    in0=iota_row[:].rearrange("p n -> p () n").broadcast_to(
                        [128, twi, 128]),
                    in1=dr_sb[:, t0i:t0i + twi].rearrange("p t -> p t ()"
                                                          ).broadcast_to(
                        [128, twi, 128]),
                    op=OP.is_equal,
                )
                s_tiles[wi] = s_t

            # two transpose/scatter windows in flight: window w's scatter
            # matmuls interleave with window w+2's pre matmuls, giving the
            # h-transpose DMA a full window of slack to land
            from collections import deque
            pend = deque()

            def flush_tiles(count):
                if not pend:
                    return
                ht_, s_w_, at_, tw_, nj_, w_, t_next = pend[0]
                t_end = min(tw_, t_next + count)
                for t in range(t_next, t_end):
                    nc.tensor.matmul(
                        out=at_[:, :nj_],
                        lhsT=ht_[:, t * 128:(t + 1) * 128],
                        rhs=s_w_[:, t * 128:t * 128 + nj_],
                        start=(t == 0), stop=(t == tw_ - 1),
                        skip_group_check=True)
                if t_end == tw_:
                    nc.vector.tensor_copy(
                        out=aggr_sb[:, w_ * win:w_ * win + nj_],
                        in_=at_[:, :nj_])
                    pend.popleft()
                    ng["drained"] = w_ + 1
                    dispatch_node_groups()
                else:
                    pend[0] = (ht_, s_w_, at_, tw_, nj_, w_, t_end)

            WCH = 2      # windows per stream-DMA chunk
            chw = maxw * WCH * 128
            chunk = {}
            build_s(0)
            for w in range(n_win):
                tw = int(TW[w])
                t0 = int(off[w])
                n0 = w * win
                nj = min(win, npc - n0)

                if w % WCH == 0:
                    c0 = int(off[w]) * 128
                    c1 = int(off[min(w + WCH, n_win)]) * 128
                    xi_c = wk.tile([128, chw], BF, tag="xi", bufs=2)
                    nc.scalar.dma_start(out=xi_c[:, :c1 - c0],
                                        in_=xi_in[:, c0:c1])
                    xj_c = wk.tile([128, chw], BF, tag="xj", bufs=2)
                    nc.scalar.dma_start(out=xj_c[:, :c1 - c0],
                                        in_=xj_in[:, c0:c1])
                    ea_c = wk.tile([ed + 1, chw], BF, tag="ea", bufs=2)
                    nc.scalar.dma_start(out=ea_c[:, :c1 - c0],
                                        in_=ea_in[:, c0:c1])
                    chunk = dict(xi=xi_c, xj=xj_c, ea=ea_c, base=c0)
                wb = t0 * 128 - chunk["base"]     # window offset in chunk
                if w + 1 < n_win:
                    build_s(w + 1)
                s_w = s_tiles[w]

                at_ps = pp.tile([128, 128], F32, tag="at", bufs=3)
                ngrp = -(-tw // grp)
                quota = -(-maxw // max(ngrp, 1))
                h_w = wk.tile([128, maxw * 128], BF, tag="h", bufs=2)
                for g in range(ngrp):
                    k0 = g * grp
                    kn = min(grp, tw - k0)
                    csl = slice(wb + k0 * 128, wb + (k0 + kn) * 128)
                    hsl = slice(k0 * 128, (k0 + kn) * 128)
                    # pre[oc, e] = W1i.T@x_i.T + W1j.T@x_j.T + W1e.T@ea.T
                    pre = pp.tile([128, grp * 128], F32, tag="pre", bufs=2)
                    nc.tensor.matmul(out=pre[:, :kn * 128], lhsT=w_sb["W1i"][:],
                                     rhs=chunk["xi"][:, csl], start=True,
                                     stop=False, skip_group_check=True)
                    nc.tensor.matmul(out=pre[:, :kn * 128], lhsT=w_sb["W1j"][:],
                                     rhs=chunk["xj"][:, csl], start=False,
                                     stop=False, skip_group_check=True)
                    nc.tensor.matmul(out=pre[:, :kn * 128], lhsT=w_sb["W1e"][:],
                                     rhs=chunk["ea"][:, csl], start=False,
                                     stop=True, skip_group_check=True)
                    nc.scalar.activation(out=h_w[:, hsl],
                                         in_=pre[:, :kn * 128], func=AF.Relu)
                    flush_tiles(quota)
                # block-transpose the window's h [oc, e] -> ht [e, oc]
                ht_w = wk.tile([128, maxw * 128], BF, tag="ht", bufs=3)
                teng = nc.sync if (w % 2 == 0) else nc.scalar
                teng.dma_start_transpose(
                    out=ht_w[:, :tw * 128].rearrange("p (k n) -> p k n", k=tw),
                    in_=h_w[:, :tw * 128])
                if len(pend) >= 2:
                    flush_tiles(maxw + 1)      # finish the oldest window
                pend.append([ht_w, s_w, at_ps, tw, nj, w, 0])
            while pend:
                flush_tiles(maxw + 1)

            # ---------- node phase ----------
            dispatch_node_groups()
            assert ng["next"] == n_nb

            # ---------- LayerNorm phase (single act-table switch) ----------
            preo_all = preo_sb[:]
            for b in range(-(-npc // 128)):
                m0 = b * 128
                mj = min(128, npc - m0)
                if True:
                    ps_t = pp.tile([128, 128], F32, tag="tr", bufs=1)
                    nc.tensor.transpose(out=ps_t[:mj, :ch],
                                        in_=preo_all[:, m0:m0 + mj],
                                        identity=ident_f[:])
                    ssum = wk.tile([128, 1], F32, tag="ssum")
                    nc.vector.tensor_reduce(out=ssum[:mj], in_=ps_t[:mj, :ch],
                                            axis=mybir.AxisListType.X,
                                            op=OP.add)
                    sqt = wk.tile([128, 128], BF, tag="sqt")
                    qsum = wk.tile([128, 1], F32, tag="qsum")
                    nc.scalar.activation(out=sqt[:mj, :ch], in_=ps_t[:mj, :ch],
                                         func=AF.Square, accum_out=qsum[:mj])
                    mu = wk.tile([128, 1], F32, tag="mu")
                    nc.vector.tensor_scalar(out=mu[:mj], in0=ssum[:mj],
                                            scalar1=1.0 / ch, scalar2=None,
                                            op0=OP.mult)
                    mu2 = wk.tile([128, 1], F32, tag="mu2")
                    nc.vector.tensor_tensor(out=mu2[:mj], in0=mu[:mj],
                                            in1=mu[:mj], op=OP.mult)
                    var = wk.tile([128, 1], F32, tag="var")
                    nc.vector.tensor_scalar(out=var[:mj], in0=qsum[:mj],
                                            scalar1=1.0 / ch, scalar2=mu2[:mj],
                                            op0=OP.mult, op1=OP.subtract)
                    sd = wk.tile([128, 1], F32, tag="sd")
                    nc.scalar.activation(out=sd[:mj], in_=var[:mj],
                                         func=AF.Sqrt, bias=eps_col[:mj])
                    rstd = wk.tile([128, 1], F32, tag="rstd")
                    nc.vector.reciprocal(out=rstd[:mj], in_=sd[:mj])
                    nrm = wk.tile([128, 128], F32, tag="nrm")
                    nc.vector.tensor_scalar(out=nrm[:mj, :ch],
                                            in0=ps_t[:mj, :ch],
                                            scalar1=mu[:mj], scalar2=rstd[:mj],
                                            op0=OP.subtract, op1=OP.mult)
                    sc = wk.tile([128, 128], F32, tag="sc")
                    nc.gpsimd.tensor_tensor(out=sc[:mj, :ch],
                                            in0=nrm[:mj, :ch],
                                            in1=w_sb["gam"][:mj, :ch],
                                            op=OP.mult)
                    outf = wk.tile([128, 128], F32, tag="outf", bufs=3)
                    nc.gpsimd.tensor_tensor(out=outf[:mj, :ch],
                                            in0=sc[:mj, :ch],
                                            in1=w_sb["bet"][:mj, :ch],
                                            op=OP.add)
                    nc.sync.dma_start(out=out_t[m0:m0 + mj, :],
                                      in_=outf[:mj, :ch])

    nc.compile()
    return nc


# --------------------------------------------------------------------------
# public entry
# --------------------------------------------------------------------------

_CACHE = {}


def kernel(x, edge_index, edge_attr, W1, b1, W2, b2, Wg, bg,
           W_ih, b_ih, W_hh, b_hh, gamma, beta, _cfg=None, _trace=None):
    if _trace is None:
        _trace = os.environ.get("GNN_TRACE", "0") == "1"
    cfg = dict(FULL_CFG if _cfg is None else _cfg)
    cfg["has_b2"] = bool(np.any(np.asarray(b2, np.float32) != 0.0))
    in_maps, meta = host_prep(x, edge_index, edge_attr, cfg)
    w = prep_weights(W1, b1, W2, b2, Wg, bg, W_ih, b_ih, W_hh, b_hh,
                     gamma, beta, cfg)
    for m in in_maps:
        m.update(w)

    key = (meta["T_total"], tuple(meta["TW"]), cfg["has_b2"])
    if key not in _CACHE:
        _CACHE.clear()
        _CACHE[key] = build_program(cfg, meta)
    nc = _CACHE[key]

    res = run_bass_kernel_spmd(nc, in_maps, list(range(cfg["n_cores"])),
                               trace=_trace)
    out = np.concatenate([res.results[c]["out"] for c in range(cfg["n_cores"])],
                         axis=0)
    kernel.last_results = res
    if _trace and res.exec_time_ns is not None:
        print(f"HW exec time: {res.exec_time_ns} ns")
        kernel.last_exec_time_ns = res.exec_time_ns
    return out.astype(np.float32)
